# revision 1
# baseline (speedup 1.0000x reference)
"""Trainium2 Bass kernel for nn_AutoCorrelation (AutoCorrelation attention, training path).

Algorithm (per core; data-parallel over batch B=8 across 8 cores):
  1. Q, K viewed as (L=1536, D=H*E=1024). Packed real DFT along L via PE matmuls
     against a baked cos/sin basis W (L x L packed: cols 0..768 = cos f, cols
     769..1535 = sin f=1..767).
  2. Cross-spectrum S[f] = sum_ch QF * conj(KF) via fused DVE
     tensor_tensor_reduce reading the DFT results directly from PSUM.
  3. mean_value = irfft(S) (packed form) via DVE multiply-reduce against the
     same resident W.
  4. AllReduce(sum) of mean_value (6KB) across the 8 cores -> shared top-7
     delay indices via the DVE max/max_index (top-8) ops.
  5. Per-core softmax weights from own mean_value at the shared indices.
  6. Roll-aggregate out[t] = sum_i w_i * v[(t+s_i) % L] as a block-circulant
     matmul: 12 distinct 128x128 weight blocks built on-device with
     iota-compares against the (runtime) shifts; out_T = sum_U Wblk[(U-T)%12]^T @ V_U.

No dynamic addressing anywhere; the data-dependent values only enter via
compare-against-scalar ops and the matmul weight blocks.
"""

import numpy as np

import concourse.bass as bass
import concourse.mybir as mybir
import concourse.tile as tile
from concourse import bacc
from concourse import bass_utils

B, L, H, E = 8, 1536, 16, 64
D = H * E            # 1024
P = 128
NC = L // P          # 12 chunks
NF = L // 2 + 1      # 769 rfft bins
TOPK = 7
F32 = mybir.dt.float32

# matmul compute dtype tag: float32r = full-rate fp32 (reduced internal
# precision), float32 = exact but 4 cycles/row.
MM_DTYPE = mybir.dt.float32r

AL = mybir.AluOpType


def _build_w_sbuf_layout() -> np.ndarray:
    """W[l, m] packed DFT basis, laid out host-side exactly as the SBUF tile:
    out[p, (mi*NC + li)*P + j] = W[li*P + p, mi*P + j],  shape (P, NC*NC*P)."""
    l = np.arange(L, dtype=np.float64)[:, None]
    f_cos = np.arange(NF, dtype=np.float64)[None, :]
    f_sin = np.arange(1, L - NF + 1, dtype=np.float64)[None, :]
    Wc = np.cos(2.0 * np.pi * l * f_cos / L)
    Ws = np.sin(2.0 * np.pi * l * f_sin / L)
    W = np.concatenate([Wc, Ws], axis=1).astype(np.float32)  # (L, L)
    # chunk to SBUF layout
    out = np.empty((P, NC * NC * P), np.float32)
    for mi in range(NC):
        for li in range(NC):
            out[:, (mi * NC + li) * P:(mi * NC + li + 1) * P] = (
                W[li * P:(li + 1) * P, mi * P:(mi + 1) * P]
            )
    return out


def _mm(ap):
    return ap.bitcast(MM_DTYPE) if MM_DTYPE != F32 else ap


def build_program(single_core: bool = False) -> bass.Bass:
    # single_core=True replaces the AllReduce with a DRAM copy (for TimelineSim)
    nc = bacc.Bacc(
        "TRN2",
        target_bir_lowering=False,
        debug=False,
        num_devices=1 if single_core else B,
        name="autocorr",
        dynamic_dma_scratch_size=512,
    )

    q_in = nc.dram_tensor("q", [L, D], F32, kind="ExternalInput")
    k_in = nc.dram_tensor("k", [L, D], F32, kind="ExternalInput")
    v_in = nc.dram_tensor("v", [L, D], F32, kind="ExternalInput")
    out_dram = nc.dram_tensor("out", [L, D], F32, kind="ExternalOutput")
    w_dram = nc.inline_tensor(_build_w_sbuf_layout(), name="wdft")

    alpha = 1.0 / (L * D)

    with tile.TileContext(nc) as tc:
        with (
            tc.tile_pool(name="misc", bufs=1) as misc,
            tc.tile_pool(name="dram", bufs=1, space="DRAM") as dram,
            tc.tile_pool(name="outp", bufs=3) as outp,
        ):
            # ---- tiles that live across phases ----
            s2d = misc.tile([P, NC], F32, tag="s2d")       # packed spectrum S'
            mv2d = misc.tile([P, NC], F32, tag="mv2d")     # own mean_value
            junk = misc.tile([P, L], F32, tag="junk")      # product scratch
            s768 = misc.tile([1, 1], F32, tag="s768")
            sbc = misc.tile([P, L], F32, tag="sbc")        # S' row-broadcast
            jk2 = misc.tile([P, D], F32, tag="jk2")        # ACT reduce dump
            bm = misc.tile([1, L], F32, tag="bm")          # batch-summed mv
            # DRAM bounces stored TRANSPOSED (c-major) so reads are contiguous
            ds1 = dram.tile([NC, P], F32)                  # S' flatten bounce
            cc_in = dram.tile([NC, P], F32)
            cc_out = dram.tile([NC, P], F32)

            with (
                tc.tile_pool(name="wpool", bufs=1) as wpool,
                tc.tile_pool(name="qkpool", bufs=1) as qkpool,
                tc.tile_pool(name="dftpsum", bufs=2, space="PSUM") as dftpsum,
            ):
                wbig = wpool.tile([P, NC * NC * P], F32, tag="wbig")
                qbig = qkpool.tile([P, NC * D], F32, tag="qbig")
                kbig = qkpool.tile([P, NC * D], F32, tag="kbig")

                # ---- loads: W[m=0] first (gates the first matmuls), then
                # Q/K (gate every pair), then remaining W in consumption order
                def w_load(mi):
                    nc.sync.dma_start(
                        _mm(wbig[:, mi * NC * P:(mi + 1) * NC * P]),
                        _mm(w_dram[:, mi * NC * P:(mi + 1) * NC * P]),
                    )
                w_load(0)
                for li in range(NC):
                    nc.sync.dma_start(
                        _mm(qbig[:, li * D:(li + 1) * D]),
                        _mm(q_in[li * P:(li + 1) * P, :]),
                    )
                    nc.sync.dma_start(
                        _mm(kbig[:, li * D:(li + 1) * D]),
                        _mm(k_in[li * P:(li + 1) * P, :]),
                    )
                for mi in [6, 1, 7, 2, 8, 3, 9, 4, 10, 5, 11]:
                    w_load(mi)

                # ---- DFT + cross-spectrum, m-chunks in Re/Im pair order ----
                qf_t: dict[int, object] = {}
                kf_t: dict[int, object] = {}
                tre = misc.tile([P, 1], F32, tag="tre")
                tim = misc.tile([P, 1], F32, tag="tim")

                # DVE reads at most one PSUM operand, so stage each KF tile
                # into SBUF (ACT engine) before the DVE multiply-reduces.
                # sbc is dead until the irfft, reuse its first D columns.
                kstage = sbc[:, 0:D]

                t2 = misc.tile([P, 1], F32, tag="t2")
                t3 = misc.tile([P, 1], F32, tag="t3")
                AX = mybir.AxisListType.X

                def mul_red(dst, a_ap, b_ap, rows=None):
                    """dst[:,0:1] = sum over free of a*b.

                    DVE does the elementwise product; the (otherwise idle)
                    ACT engine does the reduction via activation accum_out."""
                    jd = junk[:, 0:D] if rows is None else junk[0:rows, 0:D]
                    j2 = jk2[:, 0:D] if rows is None else jk2[0:rows, 0:D]
                    nc.vector.tensor_tensor(jd, a_ap, b_ap, AL.mult)
                    nc.scalar.activation(
                        out=j2, in_=jd,
                        func=mybir.ActivationFunctionType.Copy,
                        accum_out=dst,
                    )

                def emit_pair_products(r):
                    qre, qim = qf_t[r], qf_t[r + 6]
                    kre, kim = kf_t[r], kf_t[r + 6]
                    nc.scalar.copy(kstage, kre[:])
                    mul_red(tre[:, 0:1], qre[:], kstage)   # sum QRe.KRe
                    mul_red(tim[:, 0:1], qim[:], kstage)   # sum QIm.KRe
                    nc.scalar.copy(kstage, kim[:])
                    if r == 0:
                        # s768 = alpha * sum_ch QRe[768]*KRe[768] (tile 6, row 0)
                        mul_red(s768[0:1, 0:1], qim[0:1, :], kstage[0:1, :], rows=1)
                        nc.vector.tensor_scalar(
                            out=s768[0:1, 0:1], in0=s768[0:1, 0:1],
                            scalar1=alpha, scalar2=None, op0=AL.mult,
                        )
                    mul_red(t2[:, 0:1], qim[:], kstage)    # sum QIm.KIm
                    mul_red(t3[:, 0:1], qre[:], kstage)    # sum QRe.KIm
                    # S_re col r = 2a*(tre + t2);  S_im col 6+r = 2a*(tim - t3)
                    nc.vector.tensor_tensor(t2[:, 0:1], tre[:, 0:1], t2[:, 0:1], AL.add)
                    nc.vector.tensor_scalar(
                        out=s2d[:, r:r + 1], in0=t2[:, 0:1],
                        scalar1=2.0 * alpha, scalar2=None, op0=AL.mult,
                    )
                    nc.vector.tensor_tensor(
                        t3[:, 0:1], tim[:, 0:1], t3[:, 0:1], AL.subtract
                    )
                    nc.vector.tensor_scalar(
                        out=s2d[:, 6 + r:7 + r], in0=t3[:, 0:1],
                        scalar1=2.0 * alpha, scalar2=None, op0=AL.mult,
                    )
                    if r == 0:
                        # fix DC: naive col0 row0 = 2a*(Sre0 + Sre768) -> a*Sre0
                        nc.vector.tensor_scalar(
                            out=s2d[0:1, 0:1], in0=s2d[0:1, 0:1],
                            scalar1=0.5, scalar2=s768[0:1, 0:1],
                            op0=AL.mult, op1=AL.subtract,
                        )
                        # Nyquist slot (junk Im f=0): S'[768] = a*Sre768
                        nc.vector.tensor_copy(s2d[0:1, 6:7], s768[0:1, 0:1])

                m_order = [0, 6, 1, 7, 2, 8, 3, 9, 4, 10, 5, 11]
                for m in m_order:
                    qf = dftpsum.tile([P, D], F32, tag="qf")
                    kf = dftpsum.tile([P, D], F32, tag="kf")
                    for li in range(NC):
                        wv = _mm(wbig[:, (m * NC + li) * P:(m * NC + li + 1) * P])
                        st, sp = (li == 0), (li == NC - 1)
                        for nh in range(2):
                            sl = slice(nh * 512, (nh + 1) * 512)
                            nc.tensor.matmul(
                                qf[:, sl], wv,
                                _mm(qbig[:, li * D + nh * 512:li * D + (nh + 1) * 512]),
                                start=st, stop=sp,
                            )
                            nc.tensor.matmul(
                                kf[:, sl], wv,
                                _mm(kbig[:, li * D + nh * 512:li * D + (nh + 1) * 512]),
                                start=st, stop=sp,
                            )
                    qf_t[m], kf_t[m] = qf, kf
                    if m >= 6:
                        emit_pair_products(m - 6)

                # ---- irfft of own spectrum: mv2d[p, lc] ----
                # S' (P, NC) -> DRAM, then read back flattened (c p order) and
                # broadcast across all partitions: sbc[p', 128*c + p] = s2d[p, c]
                nc.sync.dma_start(ds1[:].rearrange("c p -> p c"), s2d[:])
                nc.sync.dma_start(
                    sbc[:],
                    ds1[:].rearrange("c p -> (c p)").unsqueeze(0).to_broadcast(
                        (P, L)
                    ),
                )
                wb4 = wbig[:].rearrange("p (a b c) -> p a b c", a=NC, b=NC)
                sb3 = sbc[:].rearrange("p (a c) -> p a c", a=NC)
                jk3 = junk[:].rearrange("p (a c) -> p a c", a=NC)
                jk23 = jk2[:].rearrange("p (a c) -> p a c", a=8)
                for lc in range(NC):
                    # mv2d[:, lc] = sum_pf W[t, pf] * S'[pf]; one strided DVE
                    # mult over (128, 12, 128) + one ACT accumulate.
                    nc.vector.tensor_tensor(jk3, wb4[:, :, lc, :], sb3, AL.mult)
                    nc.scalar.activation(
                        out=junk[:].rearrange("p (a c) -> p a c", a=NC), in_=jk3,
                        func=mybir.ActivationFunctionType.Copy,
                        accum_out=mv2d[:, lc:lc + 1],
                    )

            # ---- allreduce own mean_value across cores ----
            nc.sync.dma_start(cc_in[:].rearrange("c p -> p c"), mv2d[:])
            if single_core:
                nc.sync.dma_start(cc_out[:], cc_in[:])
            else:
                nc.gpsimd.collective_compute(
                    "AllReduce",
                    AL.add,
                    replica_groups=[list(range(B))],
                    ins=[cc_in[:].opt()],
                    outs=[cc_out[:].opt()],
                )
            nc.sync.dma_start(
                bm[0:1, :], cc_out[:].rearrange("c p -> (c p)").unsqueeze(0)
            )

            # ---- top-7 indices from batch-summed mean_value ----
            top8 = misc.tile([1, 8], F32, tag="top8")
            idx8 = misc.tile([1, 8], mybir.dt.uint32, tag="idx8")
            idxf = misc.tile([1, 8], F32, tag="idxf")
            nc.vector.max(top8[:], bm[0:1, :])
            nc.vector.max_index(idx8[:], top8[:], bm[0:1, :])
            nc.vector.tensor_copy(idxf[:], idx8[:])

            # ---- per-core weights: softmax(own mv at idx[0..6]) ----
            # 128-partition one-hot gathers against mv2d, then a single ones-
            # matmul for the partition reduction.
            idxd = dram.tile([1, 8], F32)
            irep = misc.tile([P, 8], F32, tag="irep")
            nc.sync.dma_start(idxd[:], idxf[0:1, :])
            nc.sync.dma_start(irep[:], idxd[0:1, :].to_broadcast((P, 8)))
            iota2d = misc.tile([P, NC], F32, tag="iota2d")
            nc.gpsimd.iota(
                iota2d[:], pattern=[[P, NC]], base=0, channel_multiplier=1,
                allow_small_or_imprecise_dtypes=True,
            )  # iota2d[p, c] = p + 128*c = flat t index
            oh2d = misc.tile([P, NC], F32, tag="oh2d")
            rgat = misc.tile([P, 8], F32, tag="rgat")
            for i in range(TOPK):
                nc.vector.tensor_scalar(
                    out=oh2d[:], in0=iota2d[:], scalar1=irep[:, i:i + 1],
                    scalar2=None, op0=AL.is_equal,
                )
                nc.vector.tensor_tensor(oh2d[:], oh2d[:], mv2d[:], AL.mult)
                nc.vector.tensor_reduce(
                    out=rgat[:, i:i + 1], in_=oh2d[:],
                    axis=mybir.AxisListType.X, op=AL.add,
                )
            ones = misc.tile([P, 1], F32, tag="ones")
            nc.vector.memset(ones[:], 1.0)
            wraw = misc.tile([1, 8], F32, tag="wraw")
            with tc.tile_pool(name="midpsum", bufs=1, space="PSUM") as midpsum:
                wps = midpsum.tile([1, 8], F32, tag="wps")
                nc.tensor.matmul(
                    wps[0:1, 0:TOPK], ones[:], rgat[:, 0:TOPK],
                    start=True, stop=True,
                )
                nc.scalar.copy(wraw[0:1, 0:TOPK], wps[0:1, 0:TOPK])
            negmax = misc.tile([1, 1], F32, tag="negmax")
            nc.vector.tensor_reduce(
                out=negmax[0:1, 0:1], in_=wraw[0:1, 0:TOPK],
                axis=mybir.AxisListType.X, op=AL.max, negate=True,
            )
            ew = misc.tile([1, 8], F32, tag="ew")
            sumw = misc.tile([1, 1], F32, tag="sumw")
            nc.scalar.activation(
                out=ew[0:1, 0:TOPK], in_=wraw[0:1, 0:TOPK],
                func=mybir.ActivationFunctionType.Exp,
                bias=negmax[0:1, 0:1], scale=1.0,
                accum_out=sumw[0:1, 0:1],
            )
            rsum = misc.tile([1, 1], F32, tag="rsum")
            nc.vector.reciprocal(rsum[0:1, 0:1], sumw[0:1, 0:1])
            wvec = misc.tile([1, 8], F32, tag="wvec")
            nc.vector.tensor_scalar(
                out=wvec[0:1, 0:TOPK], in0=ew[0:1, 0:TOPK],
                scalar1=rsum[0:1, 0:1], scalar2=None, op0=AL.mult,
            )

            # ---- v_tab: rep of shift reps per (g, i):  (1, 12*7) ----
            vt = misc.tile([1, NC * TOPK], F32, tag="vt")
            for g in range(NC):
                nc.vector.tensor_copy(
                    vt[0:1, g * TOPK:(g + 1) * TOPK], idxf[0:1, 0:TOPK]
                )
            giof = misc.tile([1, NC * TOPK], F32, tag="giof")
            nc.gpsimd.iota(
                giof[0:1, :].rearrange("o (g i) -> o g i", g=NC),
                pattern=[[-P, NC], [0, TOPK]], base=0, channel_multiplier=0,
                allow_small_or_imprecise_dtypes=True,
            )
            nc.vector.tensor_tensor(vt[:], vt[:], giof[:], AL.add)
            cwrap = misc.tile([1, NC * TOPK], F32, tag="cwrap")
            nc.vector.tensor_scalar(
                out=cwrap[:], in0=vt[:], scalar1=-768.0, scalar2=1536.0,
                op0=AL.is_lt, op1=AL.mult,
            )
            nc.vector.tensor_tensor(vt[:], vt[:], cwrap[:], AL.add)
            nc.vector.tensor_scalar(
                out=cwrap[:], in0=vt[:], scalar1=768.0, scalar2=1536.0,
                op0=AL.is_ge, op1=AL.mult,
            )
            nc.vector.tensor_tensor(vt[:], vt[:], cwrap[:], AL.subtract)

            # replicate v_tab and weights to all partitions (via DRAM bounce)
            vrep = misc.tile([P, NC * TOPK], F32, tag="vrep")
            wrep = misc.tile([P, TOPK], F32, tag="wrep")
            vtd = dram.tile([1, NC * TOPK], F32)
            wvd = dram.tile([1, TOPK], F32)
            nc.sync.dma_start(vtd[:], vt[0:1, :])
            nc.sync.dma_start(wvd[:], wvec[0:1, 0:TOPK])
            nc.sync.dma_start(vrep[:], vtd[0:1, :].to_broadcast((P, NC * TOPK)))
            nc.sync.dma_start(wrep[:], wvd[0:1, :].to_broadcast((P, TOPK)))

            # ---- build the 12 circulant weight blocks ----
            af = misc.tile([P, P], F32, tag="af")
            nc.gpsimd.iota(
                af[:], pattern=[[-1, P]], base=0, channel_multiplier=1,
                allow_small_or_imprecise_dtypes=True,
            )  # A[p, j] = p - j
            tmpw = misc.tile([P, P], F32, tag="tmpw")
            wblk = [
                misc.tile([P, P], F32, tag=f"wblk{g}", name=f"wblk{g}")
                for g in range(NC)
            ]
            tmpw2 = misc.tile([P, P], F32, tag="tmpw2")
            for g in range(NC):
                eng = nc.vector if g % 3 != 2 else nc.gpsimd
                tw = tmpw if g % 3 != 2 else tmpw2
                for i in range(TOPK):
                    dst = _mm(wblk[g][:]) if i == 0 else tw[:]
                    eng.tensor_scalar(
                        out=dst, in0=af[:],
                        scalar1=vrep[:, g * TOPK + i:g * TOPK + i + 1],
                        scalar2=wrep[:, i:i + 1],
                        op0=AL.is_equal, op1=AL.mult,
                    )
                    if i > 0:
                        eng.tensor_tensor(
                            _mm(wblk[g][:]), wblk[g][:], tw[:], AL.add
                        )

            # ---- aggregation: out_T = sum_U Wblk[(U-T)%12]^T @ V_U ----
            with (
                tc.tile_pool(name="vpool", bufs=1) as vpool,
                tc.tile_pool(name="aggpsum", bufs=4, space="PSUM") as aggpsum,
            ):
                vbig = vpool.tile([P, NC * D], F32, tag="vbig")
                for li in range(NC):
                    nc.sync.dma_start(
                        _mm(vbig[:, li * D:(li + 1) * D]),
                        _mm(v_in[li * P:(li + 1) * P, :]),
                    )
                for T in range(NC):
                    po = aggpsum.tile([P, D], F32, tag="agg")
                    for U in range(NC):
                        g = (U - T) % NC
                        st, sp = (U == 0), (U == NC - 1)
                        for nh in range(2):
                            sl = slice(nh * 512, (nh + 1) * 512)
                            nc.tensor.matmul(
                                po[:, sl], _mm(wblk[g][:]),
                                _mm(vbig[:, U * D + nh * 512:U * D + (nh + 1) * 512]),
                                start=st, stop=sp,
                            )
                    ot = outp.tile([P, D], F32, tag="ot")
                    nc.scalar.copy(ot[:], po[:])
                    nc.sync.dma_start(out_dram[T * P:(T + 1) * P, :], ot[:])

    nc.compile()
    return nc


_prog_cache = None


def _get_program():
    global _prog_cache
    if _prog_cache is None:
        _prog_cache = build_program()
    return _prog_cache


def kernel(queries, keys, values, attn_mask=0):
    nc = _get_program()
    q = np.ascontiguousarray(np.asarray(queries, dtype=np.float32).reshape(B, L, D))
    k = np.ascontiguousarray(np.asarray(keys, dtype=np.float32).reshape(B, L, D))
    v = np.ascontiguousarray(np.asarray(values, dtype=np.float32).reshape(B, L, D))
    in_maps = [{"q": q[c], "k": k[c], "v": v[c]} for c in range(B)]
    res = bass_utils.run_bass_kernel_spmd(nc, in_maps, core_ids=list(range(B)))
    out = np.stack([res.results[c]["out"] for c in range(B)])
    return out.reshape(B, L, H, E)


if __name__ == "__main__":
    prog = build_program()
    print("program built ok;", len(prog.m.functions[0].allocations), "allocations")



# revision 25
# speedup vs baseline: 1.2578x; 1.2578x over previous
"""Trainium2 Bass kernel for nn_AutoCorrelation (AutoCorrelation attention).

Algorithm (per core; data-parallel over batch B=8 across 8 cores):
  1. Q, K viewed as (L=1536, D=H*E=1024), cast to fp16 and transposed on the
     DMA XBAR into (ch-partition, time-free) layout.
  2. Direct circular cross-correlation instead of the FFT round-trip:
     corr[d] = (1/D) sum_ch sum_t k[ch,t] q[ch,(t+d)%L], computed as
     C = K^T Q via PE matmuls that accumulate the 12 time-chunk diagonal
     bands into one (128, 1536) PSUM strip G, where G[p, U] contributes to
     corr[(U - p) % L].
  3. Diagonal reduction via a skewed DRAM view: G written twice into rows of
     width 3073, read back with row stride 3074 -> rows aligned so a plain
     ones-matmul partition reduction yields corr (1, 1536).
  4. AllReduce(sum) of corr (6KB) across the 8 cores -> shared top-7 delay
     indices via DVE max/max_index.
  5. Per-core softmax weights from own corr at the shared indices.
  6. Roll-aggregate out[t] = sum_i w_i * v[(t+s_i) % L] as a block-circulant
     matmul with 12 on-device-built 128x128 weight blocks;
     out_T = sum_U Wblk[(U-T)%12]^T @ V_U, scheduled so every T consumes
     blocks in order g=0,1,2,... (waves of 4 PSUM tiles).
"""

import numpy as np

import concourse.bass as bass
import concourse.mybir as mybir
import concourse.tile as tile
from concourse import bacc
from concourse import bass_utils

B, L, H, E = 8, 1536, 16, 64
D = H * E            # 1024
P = 128
NC = L // P          # 12 time chunks
NH = D // P          # 8 channel chunks
TB = NC + 3          # qT blocks incl. 3 wrap duplicates
TOPK = 7
F32 = mybir.dt.float32
F16 = mybir.dt.float16
MM_DTYPE = mybir.dt.float32r
SKW = 2 * L + 1      # dup-row width for the skew trick

AL = mybir.AluOpType

DEBUG_DUMPS = False
BISECT_NO_V = False      # skip v loads
BISECT_NO_AGG = False    # skip weight blocks + aggregation


def _mm(ap):
    return ap.bitcast(MM_DTYPE) if MM_DTYPE != F32 else ap


def build_program(single_core: bool = False) -> bass.Bass:
    nc = bacc.Bacc(
        "TRN2",
        target_bir_lowering=False,
        debug=False,
        num_devices=1 if single_core else B,
        name="autocorr",
        dynamic_dma_scratch_size=512,
    )

    q_in = nc.dram_tensor("q", [L, D], F32, kind="ExternalInput")
    k_in = nc.dram_tensor("k", [L, D], F32, kind="ExternalInput")
    v_in = nc.dram_tensor("v", [L, D], F32, kind="ExternalInput")
    out_dram = nc.dram_tensor("out", [L, D], F32, kind="ExternalOutput")
    if DEBUG_DUMPS:
        dbg_corr = nc.dram_tensor("dbg_corr", [1, L], F32, kind="ExternalOutput")
        dbg_g = nc.dram_tensor("dbg_g", [P, L], F32, kind="ExternalOutput")
        dbg_sk = nc.dram_tensor("dbg_sk", [P, L], F32, kind="ExternalOutput")
        dbg_idx = nc.dram_tensor("dbg_idx", [1, 8], F32, kind="ExternalOutput")
        dbg_w = nc.dram_tensor("dbg_w", [1, 8], F32, kind="ExternalOutput")
        dbg_mv = nc.dram_tensor("dbg_mv", [P, NC], F32, kind="ExternalOutput")
        dbg_ones = nc.dram_tensor("dbg_ones", [P, 2], F32, kind="ExternalOutput")
        dbg_c2 = nc.dram_tensor("dbg_c2", [1, L], F32, kind="ExternalOutput")
        dbg_c3 = nc.dram_tensor("dbg_c3", [1, L], F32, kind="ExternalOutput")

    with tile.TileContext(nc) as tc:
        with (
            tc.tile_pool(name="misc", bufs=1) as misc,
            tc.tile_pool(name="dram", bufs=1, space="DRAM") as dram,
            tc.tile_pool(name="outp", bufs=3) as outp,
            tc.tile_pool(name="qkt", bufs=1) as qkt,
            tc.tile_pool(name="vpool", bufs=1) as vpool,
        ):
            qT = qkt.tile([P, TB * NH * P], F16, tag="qT")
            kT = qkt.tile([P, NC * NH * P], F16, tag="kT")
            qT4 = qT[:].rearrange("c (b h t) -> c b h t", b=TB, h=NH)
            kT4 = kT[:].rearrange("c (b h t) -> c b h t", b=NC, h=NH)
            vbig = vpool.tile([P, NC * D], F32, tag="vbig")

            # ---- phase 1: load, cast to fp16, DMA-XBAR transpose ----
            with (
                tc.tile_pool(name="stage", bufs=4) as stage,
                tc.tile_pool(name="cast", bufs=4) as castp,
                tc.tile_pool(name="gpsum", bufs=1, space="PSUM") as gpsum,
            ):
                def load_chunk(src, ci, dstT, dup=False):
                    sf = stage.tile([P, D], F32, tag="sf")
                    nc.sync.dma_start(sf[:], src[ci * P:(ci + 1) * P, :])
                    ch = castp.tile([P, D], F16, tag="ch")
                    eng = nc.vector if ci % 2 == 0 else nc.gpsimd
                    eng.tensor_copy(ch[:], sf[:])
                    nc.scalar.dma_start(dstT[:, ci, :, :], ch[:], transpose=True)
                    if dup:
                        nc.scalar.dma_start(
                            dstT[:, NC + ci, :, :], ch[:], transpose=True
                        )

                for ci in range(NC):
                    load_chunk(q_in, ci, qT4, dup=(ci < 3))
                    load_chunk(k_in, ci, kT4)
                # v loads on the scalar queue (needed only by the agg phase)
                if not BISECT_NO_V:
                    for li in range(NC):
                        nc.gpsimd.dma_start(
                            _mm(vbig[:, li * D:(li + 1) * D]),
                            _mm(v_in[li * P:(li + 1) * P, :]),
                        )

                # ---- phase 2: G strip via K^T Q matmuls ----
                # G[p, 512*gg + u] accumulates over (TI, hi):
                #   sum k[ch, 128*TI + p] * q[ch, (128*(TI+4gg) + u) % L]
                gps = gpsum.tile([P, L], F32, tag="gps")

                def dep(TI, gg):
                    s = (TI + 4 * gg) % NC
                    dq = max(2 * (b if b < NC else b - NC)
                             for b in range(s, s + 4))
                    return max(dq, 2 * TI + 1)

                steps = sorted(
                    [(TI, gg) for TI in range(NC) for gg in range(3)],
                    key=lambda x: (dep(*x), x[0], x[1]),
                )
                n_seen = [0] * 3
                n_total = [NC * NH] * 3
                for TI, gg in steps:
                    s = (TI + 4 * gg) % NC
                    for hi in range(NH):
                        st = (n_seen[gg] == 0)
                        n_seen[gg] += 1
                        sp = (n_seen[gg] == n_total[gg])
                        nc.tensor.matmul(
                            gps[:, gg * 512:(gg + 1) * 512],
                            kT4[:, TI, hi, :],
                            qT4[:, s:s + 4, hi, :],
                            start=st, stop=sp,
                        )

                # ---- phase 3: skew reduction -> corr (1, L) ----
                gsb = misc.tile([P, L], F32, tag="gsb")
                nc.scalar.copy(gsb[:, 0:768], gps[:, 0:768])
                nc.vector.tensor_copy(gsb[:, 768:1536], gps[:, 768:1536])

            fl = dram.tile([P * (SKW + 1)], F32)
            wview = fl[0:P * SKW].rearrange("(p c) -> p c", c=SKW)
            rview = fl[:].rearrange("(p c) -> p c", c=SKW + 1)
            nc.sync.dma_start(wview[:, 0:L], gsb[:])
            nc.sync.dma_start(wview[:, L:2 * L], gsb[:])
            sk = misc.tile([P, L], F32, tag="sk")
            nc.sync.dma_start(_mm(sk[:]), _mm(rview[:, 0:L]))

            ones1 = misc.tile([P, 1], F32, tag="ones1")
            nc.vector.memset(ones1[:], 1.0)
            onesD = misc.tile([P, 1], F32, tag="onesD")
            nc.vector.tensor_scalar(
                out=_mm(onesD[:]), in0=ones1[:], scalar1=1.0 / D,
                scalar2=None, op0=AL.mult,
            )
            csb = misc.tile([1, L], F32, tag="csb")
            with tc.tile_pool(name="cpsum", bufs=1, space="PSUM") as cpsum:
                cps = cpsum.tile([1, L], F32, tag="cps")
                for j in range(3):
                    nc.tensor.matmul(
                        cps[0:1, j * 512:(j + 1) * 512],
                        _mm(onesD[:]), _mm(sk[:, j * 512:(j + 1) * 512]),
                        start=True, stop=True,
                    )
                nc.scalar.copy(csb[0:1, :], cps[0:1, :])

            # ---- allreduce corr across cores; own corr -> (128, 12) ----
            cc_in = dram.tile([L], F32)
            cc_out = dram.tile([L], F32)
            nc.sync.dma_start(cc_in[:].unsqueeze(0), csb[0:1, :])
            if single_core:
                nc.sync.dma_start(cc_out[:], cc_in[:])
            else:
                nc.gpsimd.collective_compute(
                    "AllReduce",
                    AL.add,
                    replica_groups=[list(range(B))],
                    ins=[cc_in[:].opt()],
                    outs=[cc_out[:].opt()],
                )
            bm = misc.tile([1, L], F32, tag="bm")
            nc.sync.dma_start(bm[0:1, :], cc_out[:].unsqueeze(0))
            # own corr re-layout (128, 12) via PE transposes (no strided DMA)
            onesrow = misc.tile([1, P], F32, tag="onesrow")
            nc.vector.memset(onesrow[0:1, :], 1.0)
            id11 = misc.tile([1, 1], F32, tag="id11")
            nc.vector.memset(id11[0:1, 0:1], 1.0)
            mv2d = misc.tile([P, NC], F32, tag="mv2d")
            with tc.tile_pool(name="tpsum", bufs=1, space="PSUM") as tpsum:
                mvps = tpsum.tile([P, NC], F32, tag="mvps")
                for c in range(NC):
                    nc.tensor.matmul(
                        mvps[:, c:c + 1], csb[0:1, c * P:(c + 1) * P],
                        id11[0:1, 0:1], is_transpose=True,
                        start=True, stop=True,
                    )
                nc.scalar.copy(mv2d[:], mvps[:])

            # ---- top-7 indices from batch-summed corr ----
            top8 = misc.tile([1, 8], F32, tag="top8")
            idx8 = misc.tile([1, 8], mybir.dt.uint32, tag="idx8")
            idxf = misc.tile([1, 8], F32, tag="idxf")
            nc.vector.max(top8[:], bm[0:1, :])
            nc.vector.max_index(idx8[:], top8[:], bm[0:1, :])
            nc.vector.tensor_copy(idxf[:], idx8[:])
            if DEBUG_DUMPS:
                # repeat the reduction late, and with a fresh ones tile
                onesF = misc.tile([P, 1], F32, tag="onesF")
                nc.vector.memset(onesF[:], 1.0)
                onesF2 = misc.tile([P, 1], F32, tag="onesF2")
                nc.vector.tensor_scalar(
                    out=_mm(onesF2[:]), in0=onesF[:], scalar1=1.0 / D,
                    scalar2=None, op0=AL.mult,
                )
                csb2 = misc.tile([1, L], F32, tag="csb2")
                csb3 = misc.tile([1, L], F32, tag="csb3")
                with tc.tile_pool(name="c2psum", bufs=1, space="PSUM") as c2p:
                    cps2 = c2p.tile([1, L], F32, tag="cps2")
                    for j in range(3):
                        nc.tensor.matmul(
                            cps2[0:1, j * 512:(j + 1) * 512],
                            _mm(onesD[:]), _mm(sk[:, j * 512:(j + 1) * 512]),
                            start=True, stop=True,
                        )
                    nc.scalar.copy(csb2[0:1, :], cps2[0:1, :])
                    cps3 = c2p.tile([1, L], F32, tag="cps3")
                    for j in range(3):
                        nc.tensor.matmul(
                            cps3[0:1, j * 512:(j + 1) * 512],
                            _mm(onesF2[:]), _mm(sk[:, j * 512:(j + 1) * 512]),
                            start=True, stop=True,
                        )
                    nc.scalar.copy(csb3[0:1, :], cps3[0:1, :])
                nc.sync.dma_start(dbg_c2[0:1, :], csb2[0:1, :])
                nc.sync.dma_start(dbg_c3[0:1, :], csb3[0:1, :])
                nc.sync.dma_start(dbg_g[:, :], gsb[:])
                nc.sync.dma_start(dbg_ones[:, 0:1], ones1[:])
                nc.sync.dma_start(dbg_ones[:, 1:2], onesD[:])
                nc.sync.dma_start(dbg_sk[:, :], sk[:])
                nc.sync.dma_start(dbg_corr[0:1, :], csb[0:1, :])
                nc.sync.dma_start(dbg_idx[0:1, :], idxf[0:1, :])
                nc.sync.dma_start(dbg_mv[:, :], mv2d[:])

            # ---- per-core weights: softmax(own corr at idx[0..6]) ----
            # broadcast idxf to all partitions via PE ones-outer-product
            irep = misc.tile([P, 8], F32, tag="irep")
            with tc.tile_pool(name="bpsum", bufs=1, space="PSUM") as bpsum:
                irps = bpsum.tile([P, 8], F32, tag="irps")
                nc.tensor.matmul(
                    irps[:, :], onesrow[0:1, :], idxf[0:1, :],
                    start=True, stop=True,
                )
                nc.scalar.copy(irep[:], irps[:])
            iota2dg = misc.tile([P, NC], F32, tag="iota2dg")
            nc.gpsimd.iota(
                iota2dg[:], pattern=[[P, NC]], base=0, channel_multiplier=1,
                allow_small_or_imprecise_dtypes=True,
            )  # iota2d[p, c] = p + 128*c
            iota2d = misc.tile([P, NC], F32, tag="iota2d")
            nc.vector.tensor_copy(iota2d[:], iota2dg[:])
            irepv = misc.tile([P, 8], F32, tag="irepv")
            nc.vector.tensor_copy(irepv[:], irep[:])
            oh2d = misc.tile([P, NC], F32, tag="oh2d")
            rgat = misc.tile([P, 8], F32, tag="rgat")
            for i in range(TOPK):
                nc.vector.tensor_scalar(
                    out=oh2d[:], in0=iota2d[:], scalar1=irepv[:, i:i + 1],
                    scalar2=None, op0=AL.is_equal,
                )
                nc.vector.tensor_tensor(oh2d[:], oh2d[:], mv2d[:], AL.mult)
                nc.vector.tensor_reduce(
                    out=rgat[:, i:i + 1], in_=oh2d[:],
                    axis=mybir.AxisListType.X, op=AL.add,
                )
            wraw = misc.tile([1, 8], F32, tag="wraw")
            with tc.tile_pool(name="midpsum", bufs=1, space="PSUM") as midpsum:
                wps = midpsum.tile([1, 8], F32, tag="wps")
                nc.tensor.matmul(
                    wps[0:1, 0:TOPK], ones1[:], rgat[:, 0:TOPK],
                    start=True, stop=True,
                )
                nc.scalar.copy(wraw[0:1, 0:TOPK], wps[0:1, 0:TOPK])
            negmax = misc.tile([1, 1], F32, tag="negmax")
            nc.vector.tensor_reduce(
                out=negmax[0:1, 0:1], in_=wraw[0:1, 0:TOPK],
                axis=mybir.AxisListType.X, op=AL.max, negate=True,
            )
            negmax2 = misc.tile([1, 1], F32, tag="negmax2")
            nc.scalar.copy(negmax2[0:1, 0:1], negmax[0:1, 0:1])
            ew = misc.tile([1, 8], F32, tag="ew")
            sumw = misc.tile([1, 1], F32, tag="sumw")
            nc.scalar.activation(
                out=ew[0:1, 0:TOPK], in_=wraw[0:1, 0:TOPK],
                func=mybir.ActivationFunctionType.Exp,
                bias=negmax2[0:1, 0:1], scale=1.0,
                accum_out=sumw[0:1, 0:1],
            )
            rsum = misc.tile([1, 1], F32, tag="rsum")
            nc.vector.reciprocal(rsum[0:1, 0:1], sumw[0:1, 0:1])
            wvec = misc.tile([1, 8], F32, tag="wvec")
            nc.vector.tensor_scalar(
                out=wvec[0:1, 0:TOPK], in0=ew[0:1, 0:TOPK],
                scalar1=rsum[0:1, 0:1], scalar2=None, op0=AL.mult,
            )
            if DEBUG_DUMPS:
                nc.sync.dma_start(dbg_w[0:1, :], wvec[0:1, :])

            # ---- v_tab: wrapped shift reps per (g, i): (1, 12*7) ----
            giofg = misc.tile([1, NC * TOPK], F32, tag="giofg")
            nc.gpsimd.iota(
                giofg[0:1, :].rearrange("o (g i) -> o g i", g=NC),
                pattern=[[-P, NC], [0, TOPK]], base=0, channel_multiplier=0,
                allow_small_or_imprecise_dtypes=True,
            )  # giof[0, g*7+i] = -128*g
            giof = misc.tile([1, NC * TOPK], F32, tag="giof")
            nc.vector.tensor_copy(giof[0:1, :], giofg[0:1, :])
            vt = misc.tile([1, NC * TOPK], F32, tag="vt")
            for g in range(NC):
                nc.vector.tensor_copy(
                    vt[0:1, g * TOPK:(g + 1) * TOPK], idxf[0:1, 0:TOPK]
                )
            nc.vector.tensor_tensor(vt[:], vt[:], giof[:], AL.add)
            cwrap = misc.tile([1, NC * TOPK], F32, tag="cwrap")
            nc.vector.tensor_scalar(
                out=cwrap[:], in0=vt[:], scalar1=-768.0, scalar2=1536.0,
                op0=AL.is_lt, op1=AL.mult,
            )
            nc.vector.tensor_tensor(vt[:], vt[:], cwrap[:], AL.add)
            nc.vector.tensor_scalar(
                out=cwrap[:], in0=vt[:], scalar1=768.0, scalar2=1536.0,
                op0=AL.is_ge, op1=AL.mult,
            )
            nc.vector.tensor_tensor(vt[:], vt[:], cwrap[:], AL.subtract)

            # replicate v_tab and weights to all partitions via PE broadcast
            vrep = misc.tile([P, NC * TOPK], F32, tag="vrep")
            wrep = misc.tile([P, TOPK], F32, tag="wrep")
            with tc.tile_pool(name="b2psum", bufs=1, space="PSUM") as b2psum:
                vrps = b2psum.tile([P, NC * TOPK], F32, tag="vrps")
                nc.tensor.matmul(
                    vrps[:, :], onesrow[0:1, :], vt[0:1, :],
                    start=True, stop=True,
                )
                nc.scalar.copy(vrep[:], vrps[:])
                wrps = b2psum.tile([P, TOPK], F32, tag="wrps")
                nc.tensor.matmul(
                    wrps[:, :], onesrow[0:1, :], wvec[0:1, 0:TOPK],
                    start=True, stop=True,
                )
                nc.vector.tensor_copy(wrep[:, 0:TOPK], wrps[:, :])

            if BISECT_NO_AGG:
                for T in range(NC):
                    nc.sync.dma_start(out_dram[T * P:(T + 1) * P, :],
                                      sk[:, 0:D])
                return_early = True
            else:
                return_early = False
            # ---- build the 12 circulant weight blocks (g ascending) ----
            # per-engine staging of the pointer operands (vrep/wrep/af):
            # a same-queue copy makes pointer-operand races impossible.
            afp = misc.tile([P, P], F32, tag="afp")
            nc.gpsimd.iota(
                afp[:], pattern=[[-1, P]], base=0, channel_multiplier=1,
                allow_small_or_imprecise_dtypes=True,
            )  # af[p, j] = p - j
            afv = misc.tile([P, P], F32, tag="afv")
            nc.vector.tensor_copy(afv[:], afp[:])
            vrepp = misc.tile([P, NC * TOPK], F32, tag="vrepp")
            nc.gpsimd.tensor_copy(vrepp[:], vrep[:])
            wrepp = misc.tile([P, TOPK], F32, tag="wrepp")
            nc.gpsimd.tensor_copy(wrepp[:], wrep[:])
            vrepv = misc.tile([P, NC * TOPK], F32, tag="vrepv")
            nc.vector.tensor_copy(vrepv[:], vrep[:])
            wrepv = misc.tile([P, TOPK], F32, tag="wrepv")
            nc.vector.tensor_copy(wrepv[:], wrep[:])
            tmpw = misc.tile([P, P], F32, tag="tmpw")
            tmpw2 = misc.tile([P, P], F32, tag="tmpw2")
            wblk = [
                misc.tile([P, P], F32, tag=f"wblk{g}", name=f"wblk{g}")
                for g in range(NC)
            ]
            for g in range(NC if not return_early else 0):
                onv = (g % 2 == 0)
                eng = nc.vector if onv else nc.gpsimd
                tw = tmpw if onv else tmpw2
                afx = afv if onv else afp
                vrx = vrepv if onv else vrepp
                wrx = wrepv if onv else wrepp
                for i in range(TOPK):
                    dst = _mm(wblk[g][:]) if i == 0 else tw[:]
                    eng.tensor_scalar(
                        out=dst, in0=afx[:],
                        scalar1=vrx[:, g * TOPK + i:g * TOPK + i + 1],
                        scalar2=wrx[:, i:i + 1],
                        op0=AL.is_equal, op1=AL.mult,
                    )
                    if i > 0:
                        eng.tensor_tensor(
                            _mm(wblk[g][:]), wblk[g][:], tw[:], AL.add
                        )

            # ---- aggregation: out_T = sum_U Wblk[(U-T)%12]^T @ V_U ----
            # waves of 4 PSUM tiles; within a wave, blocks consumed in order
            # g = 0, 1, 2, ... so the build race stays ahead.
            with tc.tile_pool(name="aggpsum", bufs=4, space="PSUM") as aggpsum:
                for wave in range(3 if not return_early else 0):
                    pos = [aggpsum.tile([P, D], F32, tag="agg",
                                        name=f"agg{wave}_{ti}")
                           for ti in range(4)]
                    for g in range(NC):
                        for ti in range(4):
                            T = wave * 4 + ti
                            U = (T + g) % NC
                            st, sp = (g == 0), (g == NC - 1)
                            for nh in range(2):
                                sl = slice(nh * 512, (nh + 1) * 512)
                                nc.tensor.matmul(
                                    pos[ti][:, sl], _mm(wblk[g][:]),
                                    _mm(vbig[:, U * D + nh * 512:
                                             U * D + (nh + 1) * 512]),
                                    start=st, stop=sp,
                                )
                    for ti in range(4):
                        T = wave * 4 + ti
                        ot = outp.tile([P, D], F32, tag="ot")
                        nc.scalar.copy(ot[:], pos[ti][:])
                        nc.sync.dma_start(
                            out_dram[T * P:(T + 1) * P, :], ot[:]
                        )

    nc.compile()
    return nc


_prog_cache = None


def _get_program():
    global _prog_cache
    if _prog_cache is None:
        _prog_cache = build_program()
    return _prog_cache


def kernel(queries, keys, values, attn_mask=0):
    nc = _get_program()
    q = np.ascontiguousarray(np.asarray(queries, dtype=np.float32).reshape(B, L, D))
    k = np.ascontiguousarray(np.asarray(keys, dtype=np.float32).reshape(B, L, D))
    v = np.ascontiguousarray(np.asarray(values, dtype=np.float32).reshape(B, L, D))
    in_maps = [{"q": q[c], "k": k[c], "v": v[c]} for c in range(B)]
    res = bass_utils.run_bass_kernel_spmd(nc, in_maps, core_ids=list(range(B)))
    out = np.stack([res.results[c]["out"] for c in range(B)])
    return out.reshape(B, L, H, E)


if __name__ == "__main__":
    prog = build_program(single_core=True)
    print("program built ok")
